# revision 20
# baseline (speedup 1.0000x reference)
"""Trainium2 Bass kernel for nn_Decoder (attention LSTM decoder, LAS-style).

Sharding: data-parallel over batch B=128 across 8 NeuronCores (16 batch
elements per core, length-sorted snake assignment for load balance).

v2 redesign vs baseline:
- No activation-table switches: sigmoid computed as 0.5*tanh(x/2)+0.5
  (tanh/exp/identity all live in the exp_and_others table set).
- Gates reordered host-side to [i,f,o,g] so one tanh(0.5x) ACT instr
  covers i,f,o and one tanh(x) covers g (2 ACT instrs per LSTM).
- Embedding contribution of LSTM1 gates (Wih1[:, :256] @ emb + bias)
  precomputed on host for all steps, streamed per 8-step block; cuts
  LSTM1 from 7 to 5 contraction chunks.
- Energy PSUM laid out [128, slot, NTMAX] with -1e9 padding so softmax
  exp is ONE ACT instr per half + one DVE row-sum (vs 16 ACT instrs).
- Output-projection bias via one DVE add with host-broadcast bias tile.
- Attention pipelined in two 8-slot halves on separate PSUM banks.
"""

import sys

sys.path.insert(0, "/opt/trn_rl_repo")

import numpy as np
import ml_dtypes

import concourse.bass as bass
import concourse.mybir as mybir
import concourse.tile as tile
from concourse.bass_utils import run_bass_kernel_spmd
from concourse.vector_clock import ScopedClock

bf16 = ml_dtypes.bfloat16
fp16 = np.float16
FP32 = mybir.dt.float32
BF16 = mybir.dt.bfloat16
FP16 = mybir.dt.float16

# Problem constants (hardcoded per harness contract)
VOCAB = 1000
HID = 256
VAL = 128
KEY = 128
B = 128
T_ENC = 2048
T_DEC = 256
H1 = 512  # lstm1 hidden
N_CORES = 8
B_LOC = B // N_CORES  # 16
UNROLL = 16  # steps per For_i block
NVT = 8  # vocab tiles (1000 padded to 1024)
NTMAX = 16  # max 128-tiles per slot along T_enc

_tanh = mybir.ActivationFunctionType.Tanh
_exp = mybir.ActivationFunctionType.Exp

_mult = mybir.AluOpType.mult
_add = mybir.AluOpType.add
_AX = mybir.AxisListType.X


def _patch_tile_drain():
    """Walrus in this env rejects >1 sync wait on the kernel-tail Drain.
    Split the aggregated waits onto individual NoOps before the drain."""

    def _patched(self, tick_clock, wait_clock):
        nop1 = self.nc.sync.nop()
        wait_clock.add_sem_waits(nop1.ins, ScopedClock({None: tick_clock.global_clock}))
        si = nop1.ins.sync_info
        waits = list(si.on_wait) if si and si.on_wait else []
        if len(waits) > 1:
            si.on_wait = waits[:1]
            for w in waits[1:]:
                n = self.nc.sync.nop()
                nsi = n.ins.sync_info
                if nsi is None:
                    n.ins.sync_info = mybir.SyncInfo(on_wait=[w], on_update=[])
                else:
                    nsi.on_wait = list(nsi.on_wait or []) + [w]
        self.nc.sync.drain()
        self.nc.all_engine_barrier()
        popped = self.nc._tile_sem_poison_stack.pop()
        assert popped is self._sem_poison
        self.nc.clear_and_free_semaphores(list(self.sems.allocated().values()))
        self.nc.all_engine_barrier()

    tile.TileContext._drain_and_barrier = _patched


_patch_tile_drain()

TRACE = False
LAST_EXEC_NS = None


def _split_drain_waits(nc):
    """Walrus in this env rejects >1 sync wait per instruction. Split the
    waits of any multi-wait instruction onto single-wait NoOps that execute
    just before it on the same engine."""
    n = 0
    for f in nc.m.functions:
        for bb in f.blocks:
            newlist = []
            for inst in bb.instructions:
                si = getattr(inst, "sync_info", None)
                eng = getattr(inst, "engine", None)
                if (si and si.on_wait and len(si.on_wait) > 1
                        and eng is not None
                        and eng != mybir.EngineType.Unassigned):
                    waits = list(si.on_wait)
                    si.on_wait = waits[-1:]
                    for k, w in enumerate(waits[:-1]):
                        n += 1
                        newlist.append(mybir.InstNoOp(
                            name=f"{inst.name}_dw{k}", engine=eng,
                            sync_info=mybir.SyncInfo(on_wait=[w], on_update=[]),
                            bass_nofuse=True))
                newlist.append(inst)
            bb.instructions[:] = newlist
    return n


def build_program(NT, t_dec=T_DEC, unroll=UNROLL):
    """NT: list of 16 per-slot tile counts (ceil(max len in slot group /128)).
    Same program runs SPMD on all 8 cores."""
    NT = [int(x) for x in NT]

    nc = bass.Bass("TRN2", target_bir_lowering=False, debug=False,
                   enable_asserts=False, num_devices=N_CORES)

    # ---- DRAM I/O ----
    # K/V packed per slot at fixed stride NTMAX (col of slot j, tile tt at
    # (j*NTMAX+tt)*128); only the first NT[j] tiles are populated/used.
    K_d = nc.declare_dram_parameter("K", [128, B_LOC * NTMAX * 128], FP16, isOutput=False)
    V_d = nc.declare_dram_parameter("V", [128, B_LOC * NTMAX * 128], FP16, isOutput=False)
    W1_d = nc.declare_dram_parameter("W1T", [128, 5 * 2048], BF16, isOutput=False)
    W2_d = nc.declare_dram_parameter("W2T", [128, 5 * 512], BF16, isOutput=False)
    WL_d = nc.declare_dram_parameter("WLT", [128, 2 * NVT * 128], BF16, isOutput=False)
    MSK_d = nc.declare_dram_parameter("MSK", [128, B_LOC * NTMAX], BF16, isOutput=False)
    B2_d = nc.declare_dram_parameter("B2", [128, 4 * B_LOC], BF16, isOutput=False)
    ID_d = nc.declare_dram_parameter("ID", [128, 128], BF16, isOutput=False)
    BL_d = nc.declare_dram_parameter("BL", [128, NVT * B_LOC], FP32, isOutput=False)
    # Precomputed LSTM1 gate contribution from embeddings (+ bias), bf16.
    # Block-contiguous layout: row ib*128+p holds [16 chunks, unroll, 16 slots]
    # for block ib, partition p -> each block's DMA is one contiguous 512KB.
    n_blk = t_dec // unroll
    PRE_d = nc.declare_dram_parameter(
        "PRE", [n_blk * 128, 16 * unroll * B_LOC], BF16, isOutput=False)
    OUT_d = nc.declare_dram_parameter(
        "OUT", [n_blk * 128, unroll * NVT * B_LOC], FP16, isOutput=True)

    from contextlib import ExitStack
    with tile.TileContext(nc) as tc, ExitStack() as ctx:
        res = ctx.enter_context(tc.tile_pool(name="res", bufs=1))
        state = ctx.enter_context(tc.tile_pool(name="state", bufs=1))
        work = ctx.enter_context(tc.tile_pool(name="work", bufs=2))
        attp = ctx.enter_context(tc.tile_pool(name="attp", bufs=2))
        prep = ctx.enter_context(tc.tile_pool(name="prep", bufs=2))
        stgp = ctx.enter_context(tc.tile_pool(name="stgp", bufs=2))
        ps_g1 = ctx.enter_context(tc.tile_pool(name="ps_g1", bufs=1, space="PSUM"))
        ps_g2 = ctx.enter_context(tc.tile_pool(name="ps_g2", bufs=1, space="PSUM"))
        ps_ea = ctx.enter_context(tc.tile_pool(name="ps_ea", bufs=1, space="PSUM"))
        ps_eb = ctx.enter_context(tc.tile_pool(name="ps_eb", bufs=1, space="PSUM"))
        ps_cs = ctx.enter_context(tc.tile_pool(name="ps_cs", bufs=1, space="PSUM"))
        ps_wl = ctx.enter_context(tc.tile_pool(name="ps_wl", bufs=1, space="PSUM"))

        # ---- resident tiles ----
        K_sb = res.tile([128, B_LOC * NTMAX * 128], FP16)
        V_sb = res.tile([128, B_LOC * NTMAX * 128], FP16)
        W1_sb = res.tile([128, 5, 2048], BF16)
        W2_sb = res.tile([128, 5, 512], BF16)
        WL_sb = res.tile([128, 2, NVT * 128], BF16)
        MSK_sb = res.tile([128, B_LOC, NTMAX], BF16)
        B2_sb = res.tile([128, 4, B_LOC], BF16)
        ID_sb = res.tile([128, 128], BF16)
        BL_sb = res.tile([128, NVT, B_LOC], FP32)
        ONES_sb = res.tile([128, 128], FP32)

        nc.sync.dma_start(out=K_sb, in_=K_d[:, :])
        nc.sync.dma_start(out=V_sb, in_=V_d[:, :])
        nc.sync.dma_start(out=W1_sb, in_=W1_d[:, :].rearrange("p (c m) -> p c m", c=5))
        nc.sync.dma_start(out=W2_sb, in_=W2_d[:, :].rearrange("p (c m) -> p c m", c=5))
        nc.sync.dma_start(out=WL_sb, in_=WL_d[:, :].rearrange("p (c m) -> p c m", c=2))
        nc.sync.dma_start(out=MSK_sb, in_=MSK_d[:, :].rearrange("p (j t) -> p j t", j=B_LOC))
        nc.sync.dma_start(out=B2_sb, in_=B2_d[:, :].rearrange("p (m j) -> p m j", m=4))
        nc.sync.dma_start(out=BL_sb, in_=BL_d[:, :].rearrange("p (m j) -> p m j", m=NVT))
        nc.vector.memset(ONES_sb, 1.0)
        nc.sync.dma_start(out=ID_sb, in_=ID_d[:, :])

        # ---- recurrent state ----
        h1_sb = state.tile([128, 4, B_LOC], BF16)   # [H1 chunk part, chunk, slot]
        c1_sb = state.tile([128, 4, B_LOC], FP32)
        h2_sb = state.tile([128, B_LOC], BF16)      # [KEY part, slot]
        h2f_sb = state.tile([128, B_LOC], FP16)     # fp16 copy for energy rhs
        c2_sb = state.tile([128, B_LOC], FP32)
        ctx_sb = state.tile([128, B_LOC], BF16)     # [VAL part, slot]
        att_sb = state.tile([128, B_LOC, NTMAX], BF16)
        RS_sb = state.tile([128, B_LOC], FP32)      # per-slot partial row sums
        nc.vector.memset(h1_sb, 0.0)
        nc.vector.memset(c1_sb, 0.0)
        nc.vector.memset(h2_sb, 0.0)
        nc.vector.memset(h2f_sb, 0.0)
        nc.vector.memset(c2_sb, 0.0)
        nc.vector.memset(ctx_sb, 0.0)

        # energy PSUM halves; each step an identity@MSK matmul re-opens the
        # bank (start=True writes the mask incl. -1e9 pads), energy matmuls
        # then accumulate on top -> masked/pad cols give exp()==0
        ep_a = ps_ea.tile([128, 8, NTMAX], FP32)
        ep_b = ps_eb.tile([128, 8, NTMAX], FP32)
        cs = ps_cs.tile([128, 2, B_LOC], FP32)  # [:,0,:]=cxu  [:,1,:]=S

        def lstm1_mms(pre_buf, j):
            """LSTM1 gates. One identity@PRE matmul opens the PSUM bank with
            start=True (writing the embedding+bias gate precompute); all
            weight matmuls then accumulate with start=False. h1 chunks first
            (ready early), ctx chunk deferred to last."""
            g1 = ps_g1.tile([128, 16, B_LOC], FP32, tag="g1")
            nc.tensor.matmul(g1[:, :, :], ID_sb[:, :], pre_buf[:, j, :, :],
                             start=True, stop=False, skip_group_check=True)
            for m in range(16):
                for c in range(4):
                    nc.tensor.matmul(
                        g1[:, m, :], W1_sb[:, 1 + c, m * 128:(m + 1) * 128],
                        h1_sb[:, c, :], start=False, stop=False,
                        skip_group_check=True)
            for m in range(16):
                nc.tensor.matmul(
                    g1[:, m, :], W1_sb[:, 0, m * 128:(m + 1) * 128],
                    ctx_sb[:, :], start=False, stop=True,
                    skip_group_check=True)
            return g1

        def lstm1_tail(g1):
            """activations + state update for LSTM1. g-gate weight rows are
            pre-scaled x2 on host so tanh(0.5x) serves all four gates."""
            t_all = work.tile([128, 16, B_LOC], FP32, tag="t_all")
            nc.scalar.activation(t_all[:, :, :], g1[:, :, :], _tanh, scale=0.5)
            t_g = t_all[:, 12:16, :]
            s_ifo = work.tile([128, 12, B_LOC], FP32, tag="s_ifo")
            nc.vector.tensor_scalar(s_ifo[:, :, :], t_all[:, 0:12, :],
                                    0.5, 0.5, _mult, _add)
            t1 = work.tile([128, 4, B_LOC], FP32, tag="t1")
            nc.vector.tensor_mul(t1[:, :, :], s_ifo[:, 0:4, :], t_g[:, :, :])
            nc.vector.tensor_mul(c1_sb[:, :, :], s_ifo[:, 4:8, :], c1_sb[:, :, :])
            nc.vector.tensor_add(c1_sb[:, :, :], c1_sb[:, :, :], t1[:, :, :])
            tanh_c1 = work.tile([128, 4, B_LOC], FP32, tag="tanh_c1")
            nc.scalar.activation(tanh_c1[:, :, :], c1_sb[:, :, :], _tanh)
            nc.vector.tensor_mul(h1_sb[:, :, :], s_ifo[:, 8:12, :], tanh_c1[:, :, :])

        def lstm2(j):
            g2 = ps_g2.tile([128, 4, B_LOC], FP32, tag="g2")
            nc.tensor.matmul(g2[:, :, :], ID_sb[:, :], B2_sb[:, :, :],
                             start=True, stop=False, skip_group_check=True)
            # recurrent h2 chunk first (h2 ready earliest), then h1 chunks
            for m in range(4):
                nc.tensor.matmul(
                    g2[:, m, :], W2_sb[:, 4, m * 128:(m + 1) * 128],
                    h2_sb[:, :], start=False, stop=False, skip_group_check=True)
            for m in range(4):
                for c in range(4):
                    nc.tensor.matmul(
                        g2[:, m, :], W2_sb[:, c, m * 128:(m + 1) * 128],
                        h1_sb[:, c, :], start=False, stop=(c == 3),
                        skip_group_check=True)
            t2all = work.tile([128, 4, B_LOC], FP32, tag="t2all")
            nc.scalar.activation(t2all[:, :, :], g2[:, :, :], _tanh, scale=0.5)
            t_g2 = t2all[:, 3, :]
            s_ifo2 = work.tile([128, 3, B_LOC], FP32, tag="s_ifo2")
            nc.vector.tensor_scalar(s_ifo2[:, :, :], t2all[:, 0:3, :],
                                    0.5, 0.5, _mult, _add)
            t2 = work.tile([128, B_LOC], FP32, tag="t2")
            nc.vector.tensor_mul(t2[:, :], s_ifo2[:, 0, :], t_g2[:, :])
            nc.vector.tensor_mul(c2_sb[:, :], s_ifo2[:, 1, :], c2_sb[:, :])
            nc.vector.tensor_add(c2_sb[:, :], c2_sb[:, :], t2[:, :])
            tanh_c2 = work.tile([128, B_LOC], FP32, tag="tanh_c2")
            nc.scalar.activation(tanh_c2[:, :], c2_sb[:, :], _tanh)
            # h2f first: the energy matmuls wait on it
            nc.vector.tensor_mul(h2f_sb[:, :], s_ifo2[:, 2, :], tanh_c2[:, :])
            nc.vector.tensor_mul(h2_sb[:, :], s_ifo2[:, 2, :], tanh_c2[:, :])

        def attention(j):
            # two halves of 8 slots, each with its own PSUM bank
            for half, ep in ((0, ep_a), (1, ep_b)):
                j0 = half * 8
                nc.tensor.matmul(ep[:, :, :], ID_sb[:, :],
                                 MSK_sb[:, j0:j0 + 8, :],
                                 start=True, stop=False, skip_group_check=True)
                for j2 in range(j0, j0 + 8):
                    for tt in range(NT[j2]):
                        col = (j2 * NTMAX + tt) * 128
                        nc.tensor.matmul(ep[:, j2 - j0, tt:tt + 1],
                                         K_sb[:, col:col + 128],
                                         h2f_sb[:, j2:j2 + 1], start=False,
                                         stop=True, skip_group_check=True)
                nc.scalar.activation(att_sb[:, j0:j0 + 8, :], ep[:, :, :], _exp)
                nc.vector.tensor_reduce(RS_sb[:, j0:j0 + 8], att_sb[:, j0:j0 + 8, :],
                                        axis=_AX, op=_add)
            for half in range(2):
                j0 = half * 8
                for j2 in range(j0, j0 + 8):
                    ntj = NT[j2]
                    for tt in range(ntj):
                        col = (j2 * NTMAX + tt) * 128
                        nc.tensor.matmul(cs[:, 0, j2:j2 + 1],
                                         V_sb[:, col:col + 128],
                                         att_sb[:, j2, tt:tt + 1],
                                         start=(tt == 0), stop=(tt == ntj - 1))
            # S = total row sums broadcast to all partitions; ctx = cxu / S
            nc.tensor.matmul(cs[:, 1, :], ONES_sb[:, :], RS_sb[:, :],
                             start=True, stop=True)
            rS = work.tile([128, B_LOC], FP32, tag="rS")
            nc.vector.reciprocal(rS[:, :], cs[:, 1, :])
            nc.vector.tensor_mul(ctx_sb[:, :], cs[:, 0, :], rS[:, :])

        def out_proj(stg, j):
            wl = ps_wl.tile([128, NVT, B_LOC], FP32, tag="wl")
            rhsl = [h2_sb[:, :], ctx_sb[:, :]]
            for vt in range(NVT):
                for c in range(2):
                    nc.tensor.matmul(
                        wl[:, vt, :], WL_sb[:, c, vt * 128:(vt + 1) * 128],
                        rhsl[c], start=(c == 0), stop=(c == 1))
            nc.vector.tensor_add(stg[:, j, :, :], wl[:, :, :], BL_sb[:, :, :])

        hint = (mybir.EngineType.PE, mybir.EngineType.DVE,
                mybir.EngineType.Activation, mybir.EngineType.SP)
        # loop var steps by 128 rows per block so the same ds() slices both
        # block-contiguous DRAM tensors
        with tc.For_i(0, n_blk * 128, 128, hint_engines=hint) as iv:
            pre_buf = prep.tile([128, unroll, 16, B_LOC], BF16, tag="pre")
            stepw = 16 * B_LOC
            nc.sync.dma_start(
                out=pre_buf[:, 0:1, :, :],
                in_=PRE_d[bass.ds(iv, 128), 0:stepw].rearrange(
                    "p (u c s) -> p u c s", u=1, c=16))
            nc.sync.dma_start(
                out=pre_buf[:, 1:, :, :],
                in_=PRE_d[bass.ds(iv, 128), stepw:].rearrange(
                    "p (u c s) -> p u c s", u=unroll - 1, c=16))
            stg = stgp.tile([128, unroll, NVT, B_LOC], FP16, tag="stg")
            half = unroll // 2
            for j in range(unroll):
                g1 = lstm1_mms(pre_buf, j)
                if j > 0:
                    # previous step's output projection: fills the PE stall
                    # while this step's LSTM activations run on ACT/DVE
                    out_proj(stg, j - 1)
                if j == half + 1:
                    # first-half logits done; stream them out early
                    nc.sync.dma_start(
                        out=OUT_d[bass.ds(iv, 128), :unroll * NVT * B_LOC // 2]
                        .rearrange("p (u v s) -> p u v s", u=half, v=NVT),
                        in_=stg[:, 0:half, :, :])
                if j == unroll - 1:
                    # steps 4..6 out too; only step 7 remains at block end
                    cw = NVT * B_LOC
                    nc.sync.dma_start(
                        out=OUT_d[bass.ds(iv, 128), half * cw:(unroll - 1) * cw]
                        .rearrange("p (u v s) -> p u v s", u=unroll - 1 - half,
                                   v=NVT),
                        in_=stg[:, half:unroll - 1, :, :])
                lstm1_tail(g1)
                lstm2(j)
                attention(j)
            out_proj(stg, unroll - 1)
            nc.sync.dma_start(
                out=OUT_d[bass.ds(iv, 128), (unroll - 1) * NVT * B_LOC:]
                .rearrange("p (u v s) -> p u v s", u=1, v=NVT),
                in_=stg[:, unroll - 1:, :, :])

    _split_drain_waits(nc)
    return nc


def _prep_core_arrays(core, slots, NT, keys, values, lens, pre_all,
                      W1T, W2T, WLT, b2bc, blbc):
    K_a = np.zeros((128, B_LOC * NTMAX * 128), dtype=fp16)
    V_a = np.zeros((128, B_LOC * NTMAX * 128), dtype=fp16)
    M_a = np.full((128, B_LOC * NTMAX), -1e9, dtype=bf16)
    for j, gb in enumerate(slots):
        for tt in range(int(NT[j])):
            col = (j * NTMAX + tt) * 128
            t0 = tt * 128
            K_a[:, col:col + 128] = keys[t0:t0 + 128, gb, :].T.astype(fp16)
            V_a[:, col:col + 128] = values[t0:t0 + 128, gb, :].astype(fp16)
            tpos = np.arange(t0, t0 + 128)
            M_a[:, j * NTMAX + tt] = np.where(
                tpos < int(lens[gb]), 0.0, -1e9).astype(bf16)
    # (slots, T_dec, chunk, p) -> (p, chunk, t, slot) -> blocks of 8 steps:
    # (n_blk, p, chunk, u, slot) -> rows (n_blk*128, 16*8*16)
    pre_pcts = pre_all[slots].transpose(3, 2, 1, 0)  # (128,16,T_dec,16)
    n_blk = T_DEC // UNROLL
    # [p, chunk, (ib u), slot] -> [ib, p, u, chunk, slot]
    pre_a = np.ascontiguousarray(
        pre_pcts.reshape(128, 16, n_blk, UNROLL, B_LOC).transpose(2, 0, 3, 1, 4)
        .reshape(n_blk * 128, UNROLL * 16 * B_LOC)).astype(bf16)
    return {
        "K": K_a, "V": V_a, "W1T": W1T, "W2T": W2T, "WLT": WLT,
        "MSK": M_a, "B2": b2bc, "BL": blbc, "PRE": pre_a,
        "ID": np.eye(128, dtype=bf16),
    }


def _reorder_gates(W, hid):
    """torch gate order [i,f,g,o] (blocks of size hid) -> [i,f,o,g]."""
    return np.concatenate(
        [W[0:2 * hid], W[3 * hid:4 * hid], W[2 * hid:3 * hid]], axis=0)


def kernel(keys, values, lens, text, emb_table,
           Wih1, Whh1, bih1, bhh1, Wih2, Whh2, bih2, bhh2, Wlin, blin):
    keys = np.asarray(keys, np.float32)
    values = np.asarray(values, np.float32)
    lens_i = np.asarray(lens).astype(np.int64)
    text_i = np.asarray(text).astype(np.int64)

    # batch assignment: sort desc by len, snake over cores within groups of 8
    order = np.argsort(-lens_i, kind="stable")
    NT = np.zeros(B_LOC, dtype=int)
    core_slots = [[0] * B_LOC for _ in range(N_CORES)]
    for j in range(B_LOC):
        grp = order[j * N_CORES:(j + 1) * N_CORES]
        NT[j] = max(1, int(np.ceil(int(lens_i[grp[0]]) / 128)))
        for c in range(N_CORES):
            core_slots[c][j] = int(grp[c] if j % 2 == 0 else grp[N_CORES - 1 - c])

    # host precompute: params (gates reordered [i,f,o,g])
    Wih1_r = _reorder_gates(np.asarray(Wih1, np.float32), H1)
    Whh1_r = _reorder_gates(np.asarray(Whh1, np.float32), H1)
    b1_r = _reorder_gates(
        (np.asarray(bih1, np.float32) + np.asarray(bhh1, np.float32))[:, None],
        H1)[:, 0]
    Wih2_r = _reorder_gates(np.asarray(Wih2, np.float32), KEY)
    Whh2_r = _reorder_gates(np.asarray(Whh2, np.float32), KEY)
    b2_r = _reorder_gates(
        (np.asarray(bih2, np.float32) + np.asarray(bhh2, np.float32))[:, None],
        KEY)[:, 0]

    # g-gate rows (last quarter after [i,f,o,g] reorder) scaled x2 so the
    # device can use tanh(0.5x) for every gate
    gsc1 = np.ones((2048, 1), np.float32); gsc1[1536:] = 2.0
    gsc2 = np.ones((512, 1), np.float32); gsc2[384:] = 2.0
    Wih1_r = Wih1_r * gsc1; Whh1_r = Whh1_r * gsc1; b1_r = b1_r * gsc1[:, 0]
    Wih2_r = Wih2_r * gsc2; Whh2_r = Whh2_r * gsc2; b2_r = b2_r * gsc2[:, 0]
    # on-device LSTM1 contraction chunks: [ctx; h1*4] (emb handled on host)
    W1dev = np.concatenate([Wih1_r[:, 256:384], Whh1_r], axis=1)  # (2048, 640)
    W1T = np.ascontiguousarray(W1dev.T).astype(bf16).reshape(5, 128, 2048) \
        .transpose(1, 0, 2).reshape(128, 5 * 2048)
    W2 = np.concatenate([Wih2_r, Whh2_r], axis=1)  # (512, 640)
    W2T = np.ascontiguousarray(W2.T).astype(bf16).reshape(5, 128, 512) \
        .transpose(1, 0, 2).reshape(128, 5 * 512)
    WLf = np.zeros((NVT * 128, 256), np.float32)
    WLf[:VOCAB] = np.asarray(Wlin, np.float32)
    WLT = np.ascontiguousarray(
        np.ascontiguousarray(WLf.T).astype(bf16).reshape(2, 128, NVT * 128)
        .transpose(1, 0, 2).reshape(128, 2 * NVT * 128))

    W1T = np.ascontiguousarray(W1T)
    W2T = np.ascontiguousarray(W2T)

    b2bc = np.ascontiguousarray(
        np.repeat(b2_r.reshape(4, 128, 1), B_LOC, axis=2).transpose(1, 0, 2)
        .reshape(128, 4 * B_LOC)).astype(bf16)
    blv = np.zeros((NVT * 128,), np.float32)
    blv[:VOCAB] = np.asarray(blin, np.float32)
    blbc = np.ascontiguousarray(
        np.repeat(blv.reshape(NVT, 128, 1), B_LOC, axis=2).transpose(1, 0, 2)
        .reshape(128, NVT * B_LOC))

    # embedding-driven LSTM1 gate precompute (+bias), all steps, fp32
    emb_np = np.asarray(emb_table, np.float32)[text_i]  # (B, T_dec, 256)
    pre_all = emb_np @ Wih1_r[:, :256].T + b1_r  # (B, T_dec, 2048)
    pre_all = pre_all.reshape(B, T_DEC, 16, 128)  # b, s, chunk, p

    nc = build_program(list(NT))
    in_maps = [
        _prep_core_arrays(c, core_slots[c], NT, keys, values, lens_i,
                          pre_all, W1T, W2T, WLT, b2bc, blbc)
        for c in range(N_CORES)
    ]
    res = run_bass_kernel_spmd(nc, in_maps, list(range(N_CORES)), trace=TRACE)
    global LAST_EXEC_NS
    LAST_EXEC_NS = res.exec_time_ns

    n_blk = T_DEC // UNROLL
    preds = np.zeros((B, T_DEC, VOCAB), np.float32)
    for c in range(N_CORES):
        out = res.results[c]["OUT"].astype(np.float32).reshape(
            n_blk, 128, UNROLL, NVT, B_LOC)
        # -> (slot, t, vocab): [ib, p, u, vt, j] -> [j, ib*8+u, vt*128+p]
        flat = out.transpose(4, 0, 2, 3, 1).reshape(B_LOC, T_DEC, NVT * 128)
        for j in range(B_LOC):
            preds[core_slots[c][j]] = flat[j, :, :VOCAB]
    return preds


# revision 21
# speedup vs baseline: 1.0835x; 1.0835x over previous
"""Trainium2 Bass kernel for nn_Decoder (attention LSTM decoder, LAS-style).

Sharding: data-parallel over batch B=128 across 8 NeuronCores (16 batch
elements per core, length-sorted snake assignment for load balance).

v2 redesign vs baseline:
- No activation-table switches: sigmoid computed as 0.5*tanh(x/2)+0.5
  (tanh/exp/identity all live in the exp_and_others table set).
- Gates reordered host-side to [i,f,o,g] so one tanh(0.5x) ACT instr
  covers i,f,o and one tanh(x) covers g (2 ACT instrs per LSTM).
- Embedding contribution of LSTM1 gates (Wih1[:, :256] @ emb + bias)
  precomputed on host for all steps, streamed per 8-step block; cuts
  LSTM1 from 7 to 5 contraction chunks.
- Energy PSUM laid out [128, slot, NTMAX] with -1e9 padding so softmax
  exp is ONE ACT instr per half + one DVE row-sum (vs 16 ACT instrs).
- Output-projection bias via one DVE add with host-broadcast bias tile.
- Attention pipelined in two 8-slot halves on separate PSUM banks.
"""

import sys

sys.path.insert(0, "/opt/trn_rl_repo")

import numpy as np
import ml_dtypes

import concourse.bass as bass
import concourse.mybir as mybir
import concourse.tile as tile
from concourse.bass_utils import run_bass_kernel_spmd
from concourse.vector_clock import ScopedClock

bf16 = ml_dtypes.bfloat16
fp16 = np.float16
FP32 = mybir.dt.float32
BF16 = mybir.dt.bfloat16
FP16 = mybir.dt.float16

# Problem constants (hardcoded per harness contract)
VOCAB = 1000
HID = 256
VAL = 128
KEY = 128
B = 128
T_ENC = 2048
T_DEC = 256
H1 = 512  # lstm1 hidden
N_CORES = 8
B_LOC = B // N_CORES  # 16
UNROLL = 4  # steps per For_i block
NVT = 8  # vocab tiles (1000 padded to 1024)
NTMAX = 16  # max 128-tiles per slot along T_enc

_tanh = mybir.ActivationFunctionType.Tanh
_exp = mybir.ActivationFunctionType.Exp

_mult = mybir.AluOpType.mult
_add = mybir.AluOpType.add
_AX = mybir.AxisListType.X


def _patch_tile_drain():
    """Walrus in this env rejects >1 sync wait on the kernel-tail Drain.
    Split the aggregated waits onto individual NoOps before the drain."""

    def _patched(self, tick_clock, wait_clock):
        nop1 = self.nc.sync.nop()
        wait_clock.add_sem_waits(nop1.ins, ScopedClock({None: tick_clock.global_clock}))
        si = nop1.ins.sync_info
        waits = list(si.on_wait) if si and si.on_wait else []
        if len(waits) > 1:
            si.on_wait = waits[:1]
            for w in waits[1:]:
                n = self.nc.sync.nop()
                nsi = n.ins.sync_info
                if nsi is None:
                    n.ins.sync_info = mybir.SyncInfo(on_wait=[w], on_update=[])
                else:
                    nsi.on_wait = list(nsi.on_wait or []) + [w]
        self.nc.sync.drain()
        self.nc.all_engine_barrier()
        popped = self.nc._tile_sem_poison_stack.pop()
        assert popped is self._sem_poison
        self.nc.clear_and_free_semaphores(list(self.sems.allocated().values()))
        self.nc.all_engine_barrier()

    tile.TileContext._drain_and_barrier = _patched


_patch_tile_drain()

TRACE = False
LAST_EXEC_NS = None


def _split_drain_waits(nc):
    """Walrus in this env rejects >1 sync wait per instruction. Split the
    waits of any multi-wait instruction onto single-wait NoOps that execute
    just before it on the same engine."""
    n = 0
    for f in nc.m.functions:
        for bb in f.blocks:
            newlist = []
            for inst in bb.instructions:
                si = getattr(inst, "sync_info", None)
                eng = getattr(inst, "engine", None)
                if (si and si.on_wait and len(si.on_wait) > 1
                        and eng is not None
                        and eng != mybir.EngineType.Unassigned):
                    waits = list(si.on_wait)
                    si.on_wait = waits[-1:]
                    for k, w in enumerate(waits[:-1]):
                        n += 1
                        newlist.append(mybir.InstNoOp(
                            name=f"{inst.name}_dw{k}", engine=eng,
                            sync_info=mybir.SyncInfo(on_wait=[w], on_update=[]),
                            bass_nofuse=True))
                newlist.append(inst)
            bb.instructions[:] = newlist
    return n


def build_program(NT, t_dec=T_DEC, unroll=UNROLL):
    """NT: list of 16 per-slot tile counts (ceil(max len in slot group /128)).
    Same program runs SPMD on all 8 cores."""
    NT = [int(x) for x in NT]

    nc = bass.Bass("TRN2", target_bir_lowering=False, debug=False,
                   enable_asserts=False, num_devices=N_CORES)

    # ---- DRAM I/O ----
    # K/V packed per slot at fixed stride NTMAX (col of slot j, tile tt at
    # (j*NTMAX+tt)*128); only the first NT[j] tiles are populated/used.
    K_d = nc.declare_dram_parameter("K", [128, B_LOC * NTMAX * 128], FP16, isOutput=False)
    V_d = nc.declare_dram_parameter("V", [128, B_LOC * NTMAX * 128], FP16, isOutput=False)
    W1_d = nc.declare_dram_parameter("W1T", [128, 5 * 2048], BF16, isOutput=False)
    W2_d = nc.declare_dram_parameter("W2T", [128, 5 * 512], BF16, isOutput=False)
    WL_d = nc.declare_dram_parameter("WLT", [128, 2 * NVT * 128], BF16, isOutput=False)
    MSK_d = nc.declare_dram_parameter("MSK", [128, B_LOC * NTMAX], BF16, isOutput=False)
    B2_d = nc.declare_dram_parameter("B2", [128, 4 * B_LOC], BF16, isOutput=False)
    ID_d = nc.declare_dram_parameter("ID", [128, 128], BF16, isOutput=False)
    BL_d = nc.declare_dram_parameter("BL", [128, NVT * B_LOC], FP32, isOutput=False)
    # Precomputed LSTM1 gate contribution from embeddings (+ bias), bf16.
    # Block-contiguous layout: row ib*128+p holds [16 chunks, unroll, 16 slots]
    # for block ib, partition p -> each block's DMA is one contiguous 512KB.
    n_blk = t_dec // unroll
    PRE_d = nc.declare_dram_parameter(
        "PRE", [n_blk * 128, 16 * unroll * B_LOC], BF16, isOutput=False)
    OUT_d = nc.declare_dram_parameter(
        "OUT", [n_blk * 128, unroll * NVT * B_LOC], FP16, isOutput=True)

    from contextlib import ExitStack
    with tile.TileContext(nc) as tc, ExitStack() as ctx:
        res = ctx.enter_context(tc.tile_pool(name="res", bufs=1))
        state = ctx.enter_context(tc.tile_pool(name="state", bufs=1))
        work = ctx.enter_context(tc.tile_pool(name="work", bufs=2))
        attp = ctx.enter_context(tc.tile_pool(name="attp", bufs=2))
        prep = ctx.enter_context(tc.tile_pool(name="prep", bufs=2))
        stgp = ctx.enter_context(tc.tile_pool(name="stgp", bufs=2))
        ps_g1 = ctx.enter_context(tc.tile_pool(name="ps_g1", bufs=1, space="PSUM"))
        ps_g2 = ctx.enter_context(tc.tile_pool(name="ps_g2", bufs=1, space="PSUM"))
        ps_ea = ctx.enter_context(tc.tile_pool(name="ps_ea", bufs=1, space="PSUM"))
        ps_eb = ctx.enter_context(tc.tile_pool(name="ps_eb", bufs=1, space="PSUM"))
        ps_cs = ctx.enter_context(tc.tile_pool(name="ps_cs", bufs=1, space="PSUM"))
        ps_wl = ctx.enter_context(tc.tile_pool(name="ps_wl", bufs=1, space="PSUM"))

        # ---- resident tiles ----
        K_sb = res.tile([128, B_LOC * NTMAX * 128], FP16)
        V_sb = res.tile([128, B_LOC * NTMAX * 128], FP16)
        W1_sb = res.tile([128, 5, 2048], BF16)
        W2_sb = res.tile([128, 5, 512], BF16)
        WL_sb = res.tile([128, 2, NVT * 128], BF16)
        MSK_sb = res.tile([128, B_LOC, NTMAX], BF16)
        B2_sb = res.tile([128, 4, B_LOC], BF16)
        ID_sb = res.tile([128, 128], BF16)
        BL_sb = res.tile([128, NVT, B_LOC], FP32)
        ONES_sb = res.tile([128, 128], FP32)

        nc.sync.dma_start(out=K_sb, in_=K_d[:, :])
        nc.sync.dma_start(out=V_sb, in_=V_d[:, :])
        nc.sync.dma_start(out=W1_sb, in_=W1_d[:, :].rearrange("p (c m) -> p c m", c=5))
        nc.sync.dma_start(out=W2_sb, in_=W2_d[:, :].rearrange("p (c m) -> p c m", c=5))
        nc.sync.dma_start(out=WL_sb, in_=WL_d[:, :].rearrange("p (c m) -> p c m", c=2))
        nc.sync.dma_start(out=MSK_sb, in_=MSK_d[:, :].rearrange("p (j t) -> p j t", j=B_LOC))
        nc.sync.dma_start(out=B2_sb, in_=B2_d[:, :].rearrange("p (m j) -> p m j", m=4))
        nc.sync.dma_start(out=BL_sb, in_=BL_d[:, :].rearrange("p (m j) -> p m j", m=NVT))
        nc.vector.memset(ONES_sb, 1.0)
        nc.sync.dma_start(out=ID_sb, in_=ID_d[:, :])

        # ---- recurrent state ----
        h1_sb = state.tile([128, 4, B_LOC], BF16)   # [H1 chunk part, chunk, slot]
        c1_sb = state.tile([128, 4, B_LOC], FP32)
        h2_sb = state.tile([128, B_LOC], BF16)      # [KEY part, slot]
        h2f_sb = state.tile([128, B_LOC], FP16)     # fp16 copy for energy rhs
        c2_sb = state.tile([128, B_LOC], FP32)
        ctx_sb = state.tile([128, B_LOC], BF16)     # [VAL part, slot]
        att_sb = state.tile([128, B_LOC, NTMAX], BF16)
        RS_sb = state.tile([128, B_LOC], FP32)      # per-slot partial row sums
        nc.vector.memset(h1_sb, 0.0)
        nc.vector.memset(c1_sb, 0.0)
        nc.vector.memset(h2_sb, 0.0)
        nc.vector.memset(h2f_sb, 0.0)
        nc.vector.memset(c2_sb, 0.0)
        nc.vector.memset(ctx_sb, 0.0)

        # energy PSUM halves; each step an identity@MSK matmul re-opens the
        # bank (start=True writes the mask incl. -1e9 pads), energy matmuls
        # then accumulate on top -> masked/pad cols give exp()==0
        ep_a = ps_ea.tile([128, 8, NTMAX], FP32)
        ep_b = ps_eb.tile([128, 8, NTMAX], FP32)
        cs = ps_cs.tile([128, 2, B_LOC], FP32)  # [:,0,:]=cxu  [:,1,:]=S

        def lstm1_mms(pre_buf, j):
            """LSTM1 gates. One identity@PRE matmul opens the PSUM bank with
            start=True (writing the embedding+bias gate precompute); all
            weight matmuls then accumulate with start=False. h1 chunks first
            (ready early), ctx chunk deferred to last."""
            g1 = ps_g1.tile([128, 16, B_LOC], FP32, tag="g1")
            nc.tensor.matmul(g1[:, :, :], ID_sb[:, :], pre_buf[:, j, :, :],
                             start=True, stop=False, skip_group_check=True)
            for m in range(16):
                for c in range(4):
                    nc.tensor.matmul(
                        g1[:, m, :], W1_sb[:, 1 + c, m * 128:(m + 1) * 128],
                        h1_sb[:, c, :], start=False, stop=False,
                        skip_group_check=True)
            for m in range(16):
                nc.tensor.matmul(
                    g1[:, m, :], W1_sb[:, 0, m * 128:(m + 1) * 128],
                    ctx_sb[:, :], start=False, stop=True,
                    skip_group_check=True)
            return g1

        def lstm1_tail(g1):
            """activations + state update for LSTM1. g-gate weight rows are
            pre-scaled x2 on host so tanh(0.5x) serves all four gates."""
            t_all = work.tile([128, 16, B_LOC], FP32, tag="t_all")
            nc.scalar.activation(t_all[:, :, :], g1[:, :, :], _tanh, scale=0.5)
            t_g = t_all[:, 12:16, :]
            s_ifo = work.tile([128, 12, B_LOC], FP32, tag="s_ifo")
            nc.vector.tensor_scalar(s_ifo[:, :, :], t_all[:, 0:12, :],
                                    0.5, 0.5, _mult, _add)
            t1 = work.tile([128, 4, B_LOC], FP32, tag="t1")
            nc.vector.tensor_mul(t1[:, :, :], s_ifo[:, 0:4, :], t_g[:, :, :])
            nc.vector.tensor_mul(c1_sb[:, :, :], s_ifo[:, 4:8, :], c1_sb[:, :, :])
            nc.vector.tensor_add(c1_sb[:, :, :], c1_sb[:, :, :], t1[:, :, :])
            tanh_c1 = work.tile([128, 4, B_LOC], FP32, tag="tanh_c1")
            nc.scalar.activation(tanh_c1[:, :, :], c1_sb[:, :, :], _tanh)
            nc.vector.tensor_mul(h1_sb[:, :, :], s_ifo[:, 8:12, :], tanh_c1[:, :, :])

        def lstm2(j):
            g2 = ps_g2.tile([128, 4, B_LOC], FP32, tag="g2")
            nc.tensor.matmul(g2[:, :, :], ID_sb[:, :], B2_sb[:, :, :],
                             start=True, stop=False, skip_group_check=True)
            # recurrent h2 chunk first (h2 ready earliest), then h1 chunks
            for m in range(4):
                nc.tensor.matmul(
                    g2[:, m, :], W2_sb[:, 4, m * 128:(m + 1) * 128],
                    h2_sb[:, :], start=False, stop=False, skip_group_check=True)
            for m in range(4):
                for c in range(4):
                    nc.tensor.matmul(
                        g2[:, m, :], W2_sb[:, c, m * 128:(m + 1) * 128],
                        h1_sb[:, c, :], start=False, stop=(c == 3),
                        skip_group_check=True)
            t2all = work.tile([128, 4, B_LOC], FP32, tag="t2all")
            nc.scalar.activation(t2all[:, :, :], g2[:, :, :], _tanh, scale=0.5)
            t_g2 = t2all[:, 3, :]
            s_ifo2 = work.tile([128, 3, B_LOC], FP32, tag="s_ifo2")
            nc.vector.tensor_scalar(s_ifo2[:, :, :], t2all[:, 0:3, :],
                                    0.5, 0.5, _mult, _add)
            t2 = work.tile([128, B_LOC], FP32, tag="t2")
            nc.vector.tensor_mul(t2[:, :], s_ifo2[:, 0, :], t_g2[:, :])
            nc.vector.tensor_mul(c2_sb[:, :], s_ifo2[:, 1, :], c2_sb[:, :])
            nc.vector.tensor_add(c2_sb[:, :], c2_sb[:, :], t2[:, :])
            tanh_c2 = work.tile([128, B_LOC], FP32, tag="tanh_c2")
            nc.scalar.activation(tanh_c2[:, :], c2_sb[:, :], _tanh)
            # h2f first: the energy matmuls wait on it
            nc.vector.tensor_mul(h2f_sb[:, :], s_ifo2[:, 2, :], tanh_c2[:, :])
            nc.vector.tensor_mul(h2_sb[:, :], s_ifo2[:, 2, :], tanh_c2[:, :])

        def attention(j):
            # two halves of 8 slots, each with its own PSUM bank
            for half, ep in ((0, ep_a), (1, ep_b)):
                j0 = half * 8
                nc.tensor.matmul(ep[:, :, :], ID_sb[:, :],
                                 MSK_sb[:, j0:j0 + 8, :],
                                 start=True, stop=False, skip_group_check=True)
                for j2 in range(j0, j0 + 8):
                    for tt in range(NT[j2]):
                        col = (j2 * NTMAX + tt) * 128
                        nc.tensor.matmul(ep[:, j2 - j0, tt:tt + 1],
                                         K_sb[:, col:col + 128],
                                         h2f_sb[:, j2:j2 + 1], start=False,
                                         stop=True, skip_group_check=True)
                nc.scalar.activation(att_sb[:, j0:j0 + 8, :], ep[:, :, :], _exp)
                nc.vector.tensor_reduce(RS_sb[:, j0:j0 + 8], att_sb[:, j0:j0 + 8, :],
                                        axis=_AX, op=_add)
            for half in range(2):
                j0 = half * 8
                for j2 in range(j0, j0 + 8):
                    ntj = NT[j2]
                    for tt in range(ntj):
                        col = (j2 * NTMAX + tt) * 128
                        nc.tensor.matmul(cs[:, 0, j2:j2 + 1],
                                         V_sb[:, col:col + 128],
                                         att_sb[:, j2, tt:tt + 1],
                                         start=(tt == 0), stop=(tt == ntj - 1))
            # S = total row sums broadcast to all partitions; ctx = cxu / S
            nc.tensor.matmul(cs[:, 1, :], ONES_sb[:, :], RS_sb[:, :],
                             start=True, stop=True)
            rS = work.tile([128, B_LOC], FP32, tag="rS")
            nc.vector.reciprocal(rS[:, :], cs[:, 1, :])
            nc.vector.tensor_mul(ctx_sb[:, :], cs[:, 0, :], rS[:, :])

        def out_proj(stg, j):
            wl = ps_wl.tile([128, NVT, B_LOC], FP32, tag="wl")
            rhsl = [h2_sb[:, :], ctx_sb[:, :]]
            for vt in range(NVT):
                for c in range(2):
                    nc.tensor.matmul(
                        wl[:, vt, :], WL_sb[:, c, vt * 128:(vt + 1) * 128],
                        rhsl[c], start=(c == 0), stop=(c == 1))
            nc.vector.tensor_add(stg[:, j, :, :], wl[:, :, :], BL_sb[:, :, :])

        hint = (mybir.EngineType.PE, mybir.EngineType.DVE,
                mybir.EngineType.Activation, mybir.EngineType.SP)
        # loop var steps by 128 rows per block so the same ds() slices both
        # block-contiguous DRAM tensors
        with tc.For_i(0, n_blk * 128, 128, hint_engines=hint) as iv:
            pre_buf = prep.tile([128, unroll, 16, B_LOC], BF16, tag="pre")
            stepw = 16 * B_LOC
            nc.sync.dma_start(
                out=pre_buf[:, 0:1, :, :],
                in_=PRE_d[bass.ds(iv, 128), 0:stepw].rearrange(
                    "p (u c s) -> p u c s", u=1, c=16))
            nc.sync.dma_start(
                out=pre_buf[:, 1:, :, :],
                in_=PRE_d[bass.ds(iv, 128), stepw:].rearrange(
                    "p (u c s) -> p u c s", u=unroll - 1, c=16))
            stg = stgp.tile([128, unroll, NVT, B_LOC], FP16, tag="stg")
            half = unroll // 2
            for j in range(unroll):
                g1 = lstm1_mms(pre_buf, j)
                if j > 0:
                    # previous step's output projection: fills the PE stall
                    # while this step's LSTM activations run on ACT/DVE
                    out_proj(stg, j - 1)
                if j == half + 1:
                    # first-half logits done; stream them out early
                    nc.sync.dma_start(
                        out=OUT_d[bass.ds(iv, 128), :unroll * NVT * B_LOC // 2]
                        .rearrange("p (u v s) -> p u v s", u=half, v=NVT),
                        in_=stg[:, 0:half, :, :])
                if j == unroll - 1:
                    # steps 4..6 out too; only step 7 remains at block end
                    cw = NVT * B_LOC
                    nc.sync.dma_start(
                        out=OUT_d[bass.ds(iv, 128), half * cw:(unroll - 1) * cw]
                        .rearrange("p (u v s) -> p u v s", u=unroll - 1 - half,
                                   v=NVT),
                        in_=stg[:, half:unroll - 1, :, :])
                lstm1_tail(g1)
                lstm2(j)
                attention(j)
            out_proj(stg, unroll - 1)
            nc.sync.dma_start(
                out=OUT_d[bass.ds(iv, 128), (unroll - 1) * NVT * B_LOC:]
                .rearrange("p (u v s) -> p u v s", u=1, v=NVT),
                in_=stg[:, unroll - 1:, :, :])

    _split_drain_waits(nc)
    return nc


def _prep_core_arrays(core, slots, NT, keys, values, lens, pre_all,
                      W1T, W2T, WLT, b2bc, blbc):
    K_a = np.zeros((128, B_LOC * NTMAX * 128), dtype=fp16)
    V_a = np.zeros((128, B_LOC * NTMAX * 128), dtype=fp16)
    M_a = np.full((128, B_LOC * NTMAX), -1e9, dtype=bf16)
    for j, gb in enumerate(slots):
        for tt in range(int(NT[j])):
            col = (j * NTMAX + tt) * 128
            t0 = tt * 128
            K_a[:, col:col + 128] = keys[t0:t0 + 128, gb, :].T.astype(fp16)
            V_a[:, col:col + 128] = values[t0:t0 + 128, gb, :].astype(fp16)
            tpos = np.arange(t0, t0 + 128)
            M_a[:, j * NTMAX + tt] = np.where(
                tpos < int(lens[gb]), 0.0, -1e9).astype(bf16)
    # (slots, T_dec, chunk, p) -> (p, chunk, t, slot) -> blocks of 8 steps:
    # (n_blk, p, chunk, u, slot) -> rows (n_blk*128, 16*8*16)
    pre_pcts = pre_all[slots].transpose(3, 2, 1, 0)  # (128,16,T_dec,16)
    n_blk = T_DEC // UNROLL
    # [p, chunk, (ib u), slot] -> [ib, p, u, chunk, slot]
    pre_a = np.ascontiguousarray(
        pre_pcts.reshape(128, 16, n_blk, UNROLL, B_LOC).transpose(2, 0, 3, 1, 4)
        .reshape(n_blk * 128, UNROLL * 16 * B_LOC)).astype(bf16)
    return {
        "K": K_a, "V": V_a, "W1T": W1T, "W2T": W2T, "WLT": WLT,
        "MSK": M_a, "B2": b2bc, "BL": blbc, "PRE": pre_a,
        "ID": np.eye(128, dtype=bf16),
    }


def _reorder_gates(W, hid):
    """torch gate order [i,f,g,o] (blocks of size hid) -> [i,f,o,g]."""
    return np.concatenate(
        [W[0:2 * hid], W[3 * hid:4 * hid], W[2 * hid:3 * hid]], axis=0)


def kernel(keys, values, lens, text, emb_table,
           Wih1, Whh1, bih1, bhh1, Wih2, Whh2, bih2, bhh2, Wlin, blin):
    keys = np.asarray(keys, np.float32)
    values = np.asarray(values, np.float32)
    lens_i = np.asarray(lens).astype(np.int64)
    text_i = np.asarray(text).astype(np.int64)

    # batch assignment: sort desc by len, snake over cores within groups of 8
    order = np.argsort(-lens_i, kind="stable")
    NT = np.zeros(B_LOC, dtype=int)
    core_slots = [[0] * B_LOC for _ in range(N_CORES)]
    for j in range(B_LOC):
        grp = order[j * N_CORES:(j + 1) * N_CORES]
        NT[j] = max(1, int(np.ceil(int(lens_i[grp[0]]) / 128)))
        for c in range(N_CORES):
            core_slots[c][j] = int(grp[c] if j % 2 == 0 else grp[N_CORES - 1 - c])

    # host precompute: params (gates reordered [i,f,o,g])
    Wih1_r = _reorder_gates(np.asarray(Wih1, np.float32), H1)
    Whh1_r = _reorder_gates(np.asarray(Whh1, np.float32), H1)
    b1_r = _reorder_gates(
        (np.asarray(bih1, np.float32) + np.asarray(bhh1, np.float32))[:, None],
        H1)[:, 0]
    Wih2_r = _reorder_gates(np.asarray(Wih2, np.float32), KEY)
    Whh2_r = _reorder_gates(np.asarray(Whh2, np.float32), KEY)
    b2_r = _reorder_gates(
        (np.asarray(bih2, np.float32) + np.asarray(bhh2, np.float32))[:, None],
        KEY)[:, 0]

    # g-gate rows (last quarter after [i,f,o,g] reorder) scaled x2 so the
    # device can use tanh(0.5x) for every gate
    gsc1 = np.ones((2048, 1), np.float32); gsc1[1536:] = 2.0
    gsc2 = np.ones((512, 1), np.float32); gsc2[384:] = 2.0
    Wih1_r = Wih1_r * gsc1; Whh1_r = Whh1_r * gsc1; b1_r = b1_r * gsc1[:, 0]
    Wih2_r = Wih2_r * gsc2; Whh2_r = Whh2_r * gsc2; b2_r = b2_r * gsc2[:, 0]
    # on-device LSTM1 contraction chunks: [ctx; h1*4] (emb handled on host)
    W1dev = np.concatenate([Wih1_r[:, 256:384], Whh1_r], axis=1)  # (2048, 640)
    W1T = np.ascontiguousarray(W1dev.T).astype(bf16).reshape(5, 128, 2048) \
        .transpose(1, 0, 2).reshape(128, 5 * 2048)
    W2 = np.concatenate([Wih2_r, Whh2_r], axis=1)  # (512, 640)
    W2T = np.ascontiguousarray(W2.T).astype(bf16).reshape(5, 128, 512) \
        .transpose(1, 0, 2).reshape(128, 5 * 512)
    WLf = np.zeros((NVT * 128, 256), np.float32)
    WLf[:VOCAB] = np.asarray(Wlin, np.float32)
    WLT = np.ascontiguousarray(
        np.ascontiguousarray(WLf.T).astype(bf16).reshape(2, 128, NVT * 128)
        .transpose(1, 0, 2).reshape(128, 2 * NVT * 128))

    W1T = np.ascontiguousarray(W1T)
    W2T = np.ascontiguousarray(W2T)

    b2bc = np.ascontiguousarray(
        np.repeat(b2_r.reshape(4, 128, 1), B_LOC, axis=2).transpose(1, 0, 2)
        .reshape(128, 4 * B_LOC)).astype(bf16)
    blv = np.zeros((NVT * 128,), np.float32)
    blv[:VOCAB] = np.asarray(blin, np.float32)
    blbc = np.ascontiguousarray(
        np.repeat(blv.reshape(NVT, 128, 1), B_LOC, axis=2).transpose(1, 0, 2)
        .reshape(128, NVT * B_LOC))

    # embedding-driven LSTM1 gate precompute (+bias), all steps, fp32
    emb_np = np.asarray(emb_table, np.float32)[text_i]  # (B, T_dec, 256)
    pre_all = emb_np @ Wih1_r[:, :256].T + b1_r  # (B, T_dec, 2048)
    pre_all = pre_all.reshape(B, T_DEC, 16, 128)  # b, s, chunk, p

    nc = build_program(list(NT))
    in_maps = [
        _prep_core_arrays(c, core_slots[c], NT, keys, values, lens_i,
                          pre_all, W1T, W2T, WLT, b2bc, blbc)
        for c in range(N_CORES)
    ]
    res = run_bass_kernel_spmd(nc, in_maps, list(range(N_CORES)), trace=TRACE)
    global LAST_EXEC_NS
    LAST_EXEC_NS = res.exec_time_ns

    n_blk = T_DEC // UNROLL
    preds = np.zeros((B, T_DEC, VOCAB), np.float32)
    for c in range(N_CORES):
        out = res.results[c]["OUT"].astype(np.float32).reshape(
            n_blk, 128, UNROLL, NVT, B_LOC)
        # -> (slot, t, vocab): [ib, p, u, vt, j] -> [j, ib*8+u, vt*128+p]
        flat = out.transpose(4, 0, 2, 3, 1).reshape(B_LOC, T_DEC, NVT * 128)
        for j in range(B_LOC):
            preds[core_slots[c][j]] = flat[j, :, :VOCAB]
    return preds
